# revision 1
# baseline (speedup 1.0000x reference)
"""Trainium2 Bass kernel v3 for the 2-layer GCN.

Key facts driving the design (measured on this fleet):
  - dma_gather costs ~10.3 ns per descriptor (Q7 descgen-bound),
    regardless of elem size / queues / call size. ap_gather is 3x worse.
  - A full-size 8-core AllGather (1.6MB/rank) costs ~45 us; split
    collectives are catastrophically slower. Use ONE collective.
Therefore:
  - Layer 1 has a static table (dinv*z), so the host stages the
    EDGE-EXPANDED table: each (position, chunk-slot) already holds its
    source row. Layer-1 aggregation = contiguous stream + one-hot
    matmuls. Zero descriptors, no Q7.
  - Layer 2 gathers from the AllGathered g2 table with dma_gather.
    Slots are pair-deduped and parity-merged (each slot's even/odd
    node can feed different dst rows via two one-hot matmuls), cutting
    descriptor count ~8%. Optional prepare_only mode runs all of
    layer-2's descgen during layer 1 + the collective (prep=True).

Math (aggregate-first): h = relu(dinv_d*(agg1@W1)+b1),
out = dinv_d*(agg2@W2)+b2 with agg_k the plain segment-sum of the
dinv-scaled feature tables (self-loops included: layer 1 in the
expansion, layer 2 via an identity matmul of the own-block g2 tile).
"""

import numpy as np

P = 128
NC_DEFAULT = 8


def _ceil_div(a, b):
    return (a + b - 1) // b


class GCNPlan3:
    def __init__(self, z, ei, W1, b1, W2, b2, n_cores=NC_DEFAULT):
        import ml_dtypes
        bf16 = ml_dtypes.bfloat16

        N, Fin = z.shape
        F = W1.shape[1]
        E = ei.shape[1]
        self.N, self.Fin, self.F, self.NC = N, Fin, F, n_cores

        src = np.asarray(ei[0])
        dst = np.asarray(ei[1])
        deg = np.bincount(dst, minlength=N) + 1
        dinv = (1.0 / np.sqrt(deg.astype(np.float64))).astype(np.float32)

        BPC = _ceil_div(N, P * n_cores)
        NBLK = BPC * n_cores
        N_pad = NBLK * P
        self.BPC, self.NBLK, self.N_pad = BPC, NBLK, N_pad
        self.CP = BPC * P

        # ---- assign dst blocks to cores, balanced by in-edge count ----
        gblk = (dst // P).astype(np.int64)
        blk_cnt = np.bincount(gblk, minlength=NBLK)
        order = np.argsort(-blk_cnt, kind="stable")
        core_tot = np.zeros(n_cores, np.int64)
        core_blocks = [[] for _ in range(n_cores)]
        for g in order:
            avail = [c for c in range(n_cores) if len(core_blocks[c]) < BPC]
            c = min(avail, key=lambda c: core_tot[c])
            core_blocks[c].append(int(g))
            core_tot[c] += blk_cnt[g]
        perm = np.empty(N_pad, np.int64)
        for c in range(n_cores):
            for i, g in enumerate(core_blocks[c]):
                base = c * self.CP + i * P
                perm[g * P:(g + 1) * P] = base + np.arange(P)
        self.perm = perm

        psrc_all = perm[src]
        dl_all = (dst % P).astype(np.int64)

        eorder = np.argsort(gblk, kind="stable")
        g_sorted = gblk[eorder]
        bs = np.searchsorted(g_sorted, np.arange(NBLK + 1))
        psrc_s = psrc_all[eorder]
        dl_s = dl_all[eorder]

        # dense permuted node arrays
        dinv_p = np.zeros(N_pad, np.float32)
        z_p = np.zeros((N_pad, Fin), np.float32)
        dinv_p[perm[:N]] = dinv
        z_p[perm[:N]] = np.asarray(z, np.float32)
        g1 = (dinv_p[:, None] * z_p).astype(bf16)  # table values, layer 1

        # ---------------- layer 1: edge-expanded buckets ----------------
        # bucket (c, i) = edges into block + 128 self edges
        l1_src = [[None] * BPC for _ in range(n_cores)]
        l1_dl = [[None] * BPC for _ in range(n_cores)]
        for c in range(n_cores):
            for i, g in enumerate(core_blocks[c]):
                s, e = bs[g], bs[g + 1]
                own = c * self.CP + i * P + np.arange(P)
                l1_src[c][i] = np.concatenate([psrc_s[s:e], own])
                l1_dl[c][i] = np.concatenate([dl_s[s:e], np.arange(P)])
        self.K1 = [
            max(_ceil_div(len(l1_src[c][i]), P) for c in range(n_cores))
            for i in range(BPC)
        ]
        self.TCH1 = sum(self.K1)
        self.col1 = np.cumsum([0] + self.K1[:-1]).tolist()

        # expanded table [P, TCH1, Fin] and dloc1 [P, TCH1]
        self.exp = np.zeros((n_cores, P, self.TCH1, Fin), bf16)
        self.dloc1 = np.full((n_cores, P, self.TCH1), 200.0, np.float32)
        for c in range(n_cores):
            for i in range(BPC):
                c0 = self.col1[i]
                srcs, dls = l1_src[c][i], l1_dl[c][i]
                n = len(srcs)
                K = self.K1[i]
                buf = np.zeros(K * P, np.int64)
                buf[:n] = srcs
                vals = g1[buf].reshape(K, P, Fin).transpose(1, 0, 2)
                self.exp[c][:, c0:c0 + K, :] = vals
                dbuf = np.full(K * P, 200.0, np.float32)
                dbuf[:n] = dls
                self.dloc1[c][:, c0:c0 + K] = dbuf.reshape(K, P).T
                # zero out padding slots' values (src 0 is a real node)
                mask = np.zeros(K * P, bool)
                mask[n:] = True
                mvals = mask.reshape(K, P).T
                self.exp[c][:, c0:c0 + K, :][mvals] = 0

        # ---------------- layer 2: pair-deduped slots ----------------
        # per bucket: slots keyed by src pair-row; slot j of a pair routes
        # its even node to dlocE[j], odd node to dlocO[j] (200 = unused).
        slot_pair = [[None] * BPC for _ in range(n_cores)]
        slot_e = [[None] * BPC for _ in range(n_cores)]
        slot_o = [[None] * BPC for _ in range(n_cores)]
        for c in range(n_cores):
            for i, g in enumerate(core_blocks[c]):
                s, e = bs[g], bs[g + 1]
                pr = psrc_s[s:e] // 2
                par = psrc_s[s:e] % 2
                dl = dl_s[s:e]
                o = np.lexsort((par, pr))
                pr, par, dl = pr[o], par[o], dl[o]
                pairs, starts = np.unique(pr, return_index=True)
                ends = np.append(starts[1:], len(pr))
                sp, se_, so_ = [], [], []
                for pi, st, en in zip(pairs, starts, ends):
                    ev = dl[st:en][par[st:en] == 0]
                    od = dl[st:en][par[st:en] == 1]
                    m = max(len(ev), len(od))
                    for j in range(m):
                        sp.append(pi)
                        se_.append(ev[j] if j < len(ev) else 200)
                        so_.append(od[j] if j < len(od) else 200)
                slot_pair[c][i] = np.asarray(sp, np.int64)
                slot_e[c][i] = np.asarray(se_, np.float64)
                slot_o[c][i] = np.asarray(so_, np.float64)
        self.K2 = [
            max(_ceil_div(len(slot_pair[c][i]), P) for c in range(n_cores))
            for i in range(BPC)
        ]
        self.TCH2 = sum(self.K2)
        self.col2 = np.cumsum([0] + self.K2[:-1]).tolist()
        self.KMAX = max(max(self.K1), max(self.K2))

        self.gidx = np.zeros((n_cores, P, self.TCH2 * 8), np.int16)
        self.dlocE = np.full((n_cores, P, self.TCH2), 200.0, np.float32)
        self.dlocO = np.full((n_cores, P, self.TCH2), 200.0, np.float32)
        for c in range(n_cores):
            for i in range(BPC):
                c0 = self.col2[i]
                K = self.K2[i]
                n = len(slot_pair[c][i])
                buf = np.zeros(K * P, np.int64)
                buf[:n] = slot_pair[c][i]
                for k in range(K):
                    wrapped = buf[k * P:(k + 1) * P].reshape(8, 16).T
                    self.gidx[c][:, (c0 + k) * 8:(c0 + k + 1) * 8] = np.tile(
                        wrapped.astype(np.int16), (8, 1))
                for arr, dest in ((slot_e[c][i], self.dlocE),
                                  (slot_o[c][i], self.dlocO)):
                    dbuf = np.full(K * P, 200.0, np.float32)
                    dbuf[:n] = arr
                    dest[c][:, c0:c0 + K] = dbuf.reshape(K, P).T

        pad1 = self.TCH1 * P * n_cores / (E + N) - 1.0
        pad2 = self.TCH2 * P * n_cores / E - 1.0
        self.pad_frac = (pad1, pad2)

        dpc = dinv_p.reshape(n_cores, self.CP)
        self.dinvc = np.ascontiguousarray(
            dpc.reshape(n_cores, BPC, P).transpose(0, 2, 1))
        self.dinvb = np.ascontiguousarray(
            np.broadcast_to(dpc[:, None, :], (n_cores, P, self.CP)))
        self.identf = np.eye(P, dtype=np.float32)
        self.identb = np.eye(P, dtype=np.float32).astype(bf16)
        self.W1 = np.asarray(W1, np.float32)
        self.W2 = np.asarray(W2, np.float32)
        self.b1c = np.asarray(b1, np.float32).reshape(F, 1)
        self.b2c = np.asarray(b2, np.float32).reshape(F, 1)
        self.iotar = np.tile(
            np.arange(P, dtype=np.float32)[None, None, :], (P, self.KMAX, 1))
        self.bf16 = bf16


def build_bass3(plan, repeat=1, sp_limit=1024, n_queues=4, prep=False,
                prefix=10):
    """sp_limit: max idxs per gather call (single_packet when <=1024).
    prep: prepare_only descgen for layer 2, fired by per-position
    triggers so descgen runs during layer 1 + the collective."""
    import sys
    sys.path.insert(0, "/opt/trn_rl_repo")
    from concourse import bacc, mybir
    from concourse import library_config
    import concourse.tile as tile
    from concourse.tile_rust import add_dep_helper

    dt = mybir.dt
    bf = dt.bfloat16
    f32 = dt.float32
    NC = plan.NC
    BPC, CP, F, Fin = plan.BPC, plan.CP, plan.F, plan.Fin
    N_pad = plan.N_pad
    Relu = mybir.ActivationFunctionType.Relu
    is_eq = mybir.AluOpType.is_equal

    nc = bacc.Bacc(
        "TRN2", target_bir_lowering=False, debug=False, num_devices=NC,
        num_swdge_queues=max(1, n_queues),
    )

    exp_d = nc.dram_tensor("exp", [P, plan.TCH1, Fin], bf,
                           kind="ExternalInput")
    gidx_d = nc.dram_tensor("gidx", [P, plan.TCH2 * 8], dt.int16,
                            kind="ExternalInput")
    dloc1_d = nc.dram_tensor("dloc1", [P, plan.TCH1], bf,
                             kind="ExternalInput")
    dlocE_d = nc.dram_tensor("dlocE", [P, plan.TCH2], bf,
                             kind="ExternalInput")
    dlocO_d = nc.dram_tensor("dlocO", [P, plan.TCH2], bf,
                             kind="ExternalInput")
    iotar_d = nc.dram_tensor("iotar", [P, plan.KMAX, P], bf,
                             kind="ExternalInput")
    dinvc_d = nc.dram_tensor("dinvc", [P, BPC], f32, kind="ExternalInput")
    dinvb_d = nc.dram_tensor("dinvb", [P, CP], f32, kind="ExternalInput")
    W1_d = nc.dram_tensor("W1", [Fin, F], f32, kind="ExternalInput")
    W2_d = nc.dram_tensor("W2", [F, F], f32, kind="ExternalInput")
    b1_d = nc.dram_tensor("b1c", [F, 1], f32, kind="ExternalInput")
    b2_d = nc.dram_tensor("b2c", [F, 1], f32, kind="ExternalInput")
    identf_d = nc.dram_tensor("identf", [P, P], f32, kind="ExternalInput")
    identb_d = nc.dram_tensor("identb", [P, P], bf, kind="ExternalInput")
    out_d = nc.dram_tensor("out", [CP, F], f32, kind="ExternalOutput")

    g2loc = nc.dram_tensor("g2loc", [CP, F], bf)
    g2full = nc.dram_tensor("g2full", [N_pad // 2, 2 * F], bf,
                            addr_space="Shared")
    rg = [list(range(NC))]
    xbufs = (prefix + 3) if prep else 3

    with tile.TileContext(nc) as tc:
        with (
            tc.tile_pool(name="const", bufs=1) as cpool,
            tc.tile_pool(name="g1", bufs=3) as g1pool,
            tc.tile_pool(name="g2", bufs=xbufs) as g2pool,
            tc.tile_pool(name="oh", bufs=2) as ohpool,
            tc.tile_pool(name="ep", bufs=4) as eppool,
            tc.tile_pool(name="psum", bufs=2, space="PSUM") as pspool,
        ):
            lib = nc.gpsimd.load_library(library_config.mlp)

            gidx_sb = cpool.tile([P, plan.TCH2 * 8], dt.int16)
            nc.sync.dma_start(gidx_sb[:], gidx_d[:])
            dloc1_sb = cpool.tile([P, plan.TCH1], bf)
            nc.sync.dma_start(dloc1_sb[:], dloc1_d[:])
            dlocE_sb = cpool.tile([P, plan.TCH2], bf)
            nc.sync.dma_start(dlocE_sb[:], dlocE_d[:])
            dlocO_sb = cpool.tile([P, plan.TCH2], bf)
            nc.sync.dma_start(dlocO_sb[:], dlocO_d[:])
            iotar_sb = cpool.tile([P, plan.KMAX, P], bf)
            nc.sync.dma_start(iotar_sb[:], iotar_d[:])
            dinvc_sb = cpool.tile([P, BPC], f32)
            nc.sync.dma_start(dinvc_sb[:], dinvc_d[:])
            dinvb_sb = cpool.tile([P, CP], f32)
            nc.sync.dma_start(dinvb_sb[:], dinvb_d[:])
            W1_sb = cpool.tile([Fin, F], f32)
            nc.sync.dma_start(W1_sb[:], W1_d[:])
            W2_sb = cpool.tile([F, F], f32)
            nc.sync.dma_start(W2_sb[:], W2_d[:])
            b1_sb = cpool.tile([F, 1], f32)
            nc.sync.dma_start(b1_sb[:], b1_d[:])
            b2_sb = cpool.tile([F, 1], f32)
            nc.sync.dma_start(b2_sb[:], b2_d[:])
            identf_sb = cpool.tile([P, P], f32)
            nc.sync.dma_start(identf_sb[:], identf_d[:])
            identb_sb = cpool.tile([P, P], bf)
            nc.sync.dma_start(identb_sb[:], identb_d[:])
            g2own = cpool.tile([P, BPC, F], bf)

            def onehot(dloc_sb, c0, ncols, tag):
                oh = ohpool.tile([P, ncols, P], bf, tag=tag)
                nc.vector.tensor_tensor(
                    out=oh[:, :, :],
                    in0=dloc_sb[:, c0:c0 + ncols].to_broadcast([P, ncols, P]),
                    in1=iotar_sb[:, 0:ncols, :],
                    op=is_eq,
                )
                return oh

            def pieces(K):
                if sp_limit <= 0:
                    return [(0, K)]
                step = max(1, sp_limit // P)
                return [(k, min(step, K - k)) for k in range(0, K, step)]

            prep_sems = ([nc.alloc_semaphore(f"prep_dma_sem{q}")
                          for q in range(max(1, n_queues))] if prep else None)
            # transfer-completion counts per queue (persist across reps)
            pcnt = [0] * max(1, n_queues)

            def body():
                qrr = [0]
                # ---- layer-2 preps (descgen first; fires later) ----
                l2_preps = [None] * BPC
                l2_xg = [None] * BPC
                l2_thresh = [None] * BPC
                pend = [0] * max(1, n_queues)
                rep_start_cnt = list(pcnt)

                def emit_prep(i):
                    K = plan.K2[i]
                    if not K:
                        return
                    c0 = plan.col2[i]
                    q = qrr[0] % max(1, n_queues)
                    qrr[0] += 1
                    pend[q] += 1
                    Xg = g2pool.tile([P, K, 2 * F], bf, tag="Xg2")
                    l2_xg[i] = Xg
                    for (k0, kq) in pieces(K):
                        g = nc.gpsimd.dma_gather(
                            Xg[:, k0:k0 + kq, :],
                            g2full[:, :],
                            gidx_sb[:, (c0 + k0) * 8:(c0 + k0 + kq) * 8],
                            kq * P,
                            kq * P,
                            2 * F,
                            single_packet=(kq * P <= 1024),
                            prepare_only=True,
                            sem=prep_sems[q],
                            queue_num=q,
                        )
                        add_dep_helper(lib.ins, g.ins, sync=True,
                                       reason="lib before gather")
                        pcnt[q] += 1
                    l2_preps[i] = q
                    l2_thresh[i] = 16 * pcnt[q]

                def fire_prep(i, cc):
                    q = l2_preps[i]
                    if q is not None and pend[q] > 0:
                        t = nc.gpsimd.trigger_dma(count=None, queue_num=q)
                        add_dep_helper(t.ins, cc.ins, sync=True,
                                       reason="cc before transfers")
                        pend[q] = 0

                if prep:
                    for i in range(min(prefix, BPC)):
                        emit_prep(i)

                # ---- layer 1 ----
                for i in range(BPC):
                    K = plan.K1[i]
                    c0 = plan.col1[i]
                    Xg = g1pool.tile([P, K, Fin], bf, tag="Xg1")
                    nc.sync.dma_start(Xg[:], exp_d[:, c0:c0 + K, :])
                    oh = onehot(dloc1_sb, c0, K, "oh1")
                    ps = pspool.tile([Fin, P], f32, space="PSUM", tag="ps1")
                    for j in range(K):
                        nc.tensor.matmul(
                            ps[:], lhsT=Xg[:, j, :], rhs=oh[:, j, :],
                            start=(j == 0), stop=(j == K - 1),
                        )
                    s1 = eppool.tile([Fin, P], f32, tag="s1")
                    nc.vector.tensor_copy(s1[:], ps[:])
                    ps2 = pspool.tile([F, P], f32, space="PSUM", tag="ps2")
                    nc.tensor.matmul(ps2[:], lhsT=W1_sb[:], rhs=s1[:],
                                     start=True, stop=True)
                    t1 = eppool.tile([F, P], f32, tag="t1")
                    nc.vector.tensor_mul(
                        t1[:], ps2[:], dinvb_sb[:, i * P:(i + 1) * P])
                    a1 = eppool.tile([F, P], f32, tag="a1")
                    nc.scalar.activation(a1[:], t1[:], Relu,
                                         bias=b1_sb[:, 0:1])
                    ps3 = pspool.tile([P, F], f32, space="PSUM", tag="ps3")
                    nc.tensor.matmul(ps3[:], lhsT=a1[:], rhs=identf_sb[:],
                                     start=True, stop=True)
                    nc.vector.tensor_scalar_mul(
                        g2own[:, i, :], ps3[:], dinvc_sb[:, i:i + 1])
                    nc.sync.dma_start(g2loc[i * P:(i + 1) * P, :],
                                      g2own[:, i, :])

                # ---- collective ----
                if prep and max(rep_start_cnt) > 0:
                    # WAR: previous rep's transfers must finish reading
                    # g2full before this rep's collective overwrites it
                    for q in range(max(1, n_queues)):
                        if rep_start_cnt[q]:
                            nc.gpsimd.wait_ge(prep_sems[q],
                                              16 * rep_start_cnt[q])
                cc = nc.gpsimd.collective_compute(
                    "AllGather", mybir.AluOpType.bypass,
                    ins=[g2loc[:]], outs=[g2full[:]], replica_groups=rg,
                )

                # ---- layer 2 ----
                def agg2(i):
                    K = plan.K2[i]
                    c0 = plan.col2[i]
                    ps = pspool.tile([F, P], f32, space="PSUM", tag="ps1")
                    nc.tensor.matmul(ps[:], lhsT=g2own[:, i, :],
                                     rhs=identb_sb[:], start=True,
                                     stop=(K == 0))
                    if K:
                        if prep:
                            Xg = l2_xg[i]
                            nc.tensor.wait_ge(prep_sems[l2_preps[i]],
                                              l2_thresh[i])
                        else:
                            Xg = g2pool.tile([P, K, 2 * F], bf, tag="Xg2")
                            for (k0, kq) in pieces(K):
                                g = nc.gpsimd.dma_gather(
                                    Xg[:, k0:k0 + kq, :],
                                    g2full[:, :],
                                    gidx_sb[:, (c0 + k0) * 8:
                                            (c0 + k0 + kq) * 8],
                                    kq * P,
                                    kq * P,
                                    2 * F,
                                    single_packet=(kq * P <= 1024),
                                    queue_num=qrr[0] % max(1, n_queues),
                                )
                                qrr[0] += 1
                                add_dep_helper(lib.ins, g.ins, sync=True,
                                               reason="lib before gather")
                        ohE = onehot(dlocE_sb, c0, K, "ohE")
                        ohO = onehot(dlocO_sb, c0, K, "ohO")
                        for j in range(K):
                            nc.tensor.matmul(
                                ps[:], lhsT=Xg[:, j, 0:F], rhs=ohE[:, j, :],
                                start=False, stop=False,
                            )
                            nc.tensor.matmul(
                                ps[:], lhsT=Xg[:, j, F:2 * F],
                                rhs=ohO[:, j, :],
                                start=False, stop=(j == K - 1),
                            )
                    s2 = eppool.tile([F, P], f32, tag="s1")
                    nc.vector.tensor_copy(s2[:], ps[:])
                    ps2 = pspool.tile([F, P], f32, space="PSUM", tag="ps2")
                    nc.tensor.matmul(ps2[:], lhsT=W2_sb[:], rhs=s2[:],
                                     start=True, stop=True)
                    t2 = eppool.tile([F, P], f32, tag="t1")
                    nc.vector.tensor_mul(
                        t2[:], ps2[:], dinvb_sb[:, i * P:(i + 1) * P])
                    a2 = eppool.tile([F, P], f32, tag="a1")
                    nc.vector.tensor_scalar_add(a2[:], t2[:], b2_sb[:, 0:1])
                    ps3 = pspool.tile([P, F], f32, space="PSUM", tag="ps3")
                    nc.tensor.matmul(ps3[:], lhsT=a2[:], rhs=identf_sb[:],
                                     start=True, stop=True)
                    o = eppool.tile([P, F], f32, tag="o")
                    nc.vector.tensor_copy(o[:], ps3[:])
                    nc.sync.dma_start(out_d[i * P:(i + 1) * P, :], o[:])

                if prep:
                    for i in range(BPC):
                        if i + prefix < BPC:
                            emit_prep(i + prefix)
                        fire_prep(i, cc)
                        agg2(i)
                else:
                    for i in range(BPC):
                        agg2(i)

            for _ in range(repeat):
                body()

    nc.compile()
    return nc


def make_in_maps3(plan):
    maps = []
    for c in range(plan.NC):
        maps.append({
            "exp": plan.exp[c],
            "gidx": plan.gidx[c],
            "dloc1": plan.dloc1[c].astype(plan.bf16),
            "dlocE": plan.dlocE[c].astype(plan.bf16),
            "dlocO": plan.dlocO[c].astype(plan.bf16),
            "iotar": plan.iotar.astype(plan.bf16),
            "dinvc": plan.dinvc[c],
            "dinvb": plan.dinvb[c],
            "W1": plan.W1,
            "W2": plan.W2,
            "b1c": plan.b1c,
            "b2c": plan.b2c,
            "identf": plan.identf,
            "identb": plan.identb,
        })
    return maps


def _ensure_devices(n_cores):
    import jax

    if len(jax.devices()) >= n_cores:
        return
    try:
        jax.config.update("jax_platforms", "")
        jax.extend.backend.clear_backends()
    except Exception:
        pass
    assert len(jax.devices()) >= n_cores


_CACHE = {}


def _run(z, ei, W1, b1, W2, b2, n_cores=NC_DEFAULT, **kw):
    import sys
    sys.path.insert(0, "/opt/trn_rl_repo")
    from concourse.bass_utils import run_bass_kernel_spmd

    _ensure_devices(n_cores)
    plan = GCNPlan3(z, ei, W1, b1, W2, b2, n_cores=n_cores)
    key = (plan.N, plan.TCH1, plan.TCH2, tuple(sorted(kw.items())))
    if key not in _CACHE:
        _CACHE[key] = build_bass3(plan, **kw)
    nc = _CACHE[key]
    in_maps = make_in_maps3(plan)
    res = run_bass_kernel_spmd(nc, in_maps, list(range(n_cores)))
    outp = np.concatenate(
        [res.results[c]["out"] for c in range(plan.NC)], axis=0)
    out = outp[plan.perm[:plan.N]].astype(np.float32)
    return out, plan


def kernel(z, ei, W1, b1, W2, b2):
    out, _ = _run(np.asarray(z), np.asarray(ei), np.asarray(W1),
                  np.asarray(b1), np.asarray(W2), np.asarray(b2))
    return out

